# revision 22
# baseline (speedup 1.0000x reference)
"""Causal linear attention (fast_transformers style) on 8 Trainium2 cores.

query (8, 512, 64, 128) f32. Data-parallel: one batch element per core.
Per (batch, node) sequence of L=512 tokens: project q/k/v with 128x128
weights, phi(x)=elu(x)+1, causal linear attention via chunked scan
(C=128 intra-chunk masked matmul + inter-chunk running KV state).

Wire strategy: the axon tunnel serializes each client connection at
~40-50 MiB/s (half-duplex), but the cap is PER CONNECTION — N worker
subprocesses with their own PJRT clients scale aggregate bandwidth
nearly linearly. Data goes int8 with per-token scales both directions,
packed into ONE int8 blob per core per slice (f16 regions via bitcast).
The single host CPU is the other scarce resource, so the parent does
ALL quantization, writing blobs straight into shared memory breadth-
first across workers (so every connection starts uploading early),
while the workers stay thin: dispatch -> fetch -> dequantize into the
shared result. Matmuls run fp16 with fp32 PSUM accumulation; mask
constants and output zero-buffers stay device-resident.
"""

import os
import sys
import time

import numpy as np

HEADS = 8
E = 16
EPS = 1e-6
L = 512
NSEQ = 64
F = 128
CH = HEADS * E  # 128 output channels
C = 128         # time chunk
NC = L // C
W17 = 17 * HEADS  # 136: per-head [num(16) | den(1)] interleaved width
AMAX_FLOOR = 1e-4

NWORK = 4              # worker subprocesses (one tunnel connection each)
ND = 8 // NWORK        # cores (= batches) per worker
S = 2                  # node-axis slices pipelined per worker call
NS = NSEQ // S         # nodes per core per slice-call
NCOL = NS * NC         # scale columns per slice: one per (n, c) tile

# int8 blob row layout (rows x 128 bytes), input side
R_X = L * NS                         # xq int8 rows, row = t*NS + n
R_SC = C * NCOL * 2 // F             # f16 scales region rows
R_W = (F * CH * 2) // F              # one f16 weight matrix region rows
R_B = (3 * CH * 2 + F - 1) // F      # three f16 biases region rows
R_IN = R_X + R_SC
R_WB = 3 * R_W + R_B                 # separate weight blob, uploaded once/call
R_OUT = R_X + R_SC                   # y8 rows + f16 out-scales rows

NW_F32 = 3 * F * CH + 3 * CH         # packed weights+biases f32 words in shm


def build_nc(num_devices, debug=False):
    """Build the per-core Bass module for one slice-call (NS node seqs)."""
    from contextlib import ExitStack

    import concourse.bacc as bacc
    import concourse.mybir as mybir
    import concourse.tile as tile

    i8 = mybir.dt.int8
    f16 = mybir.dt.float16
    f32 = mybir.dt.float32
    Relu = mybir.ActivationFunctionType.Relu
    Exp = mybir.ActivationFunctionType.Exp
    AluOp = mybir.AluOpType
    AX = mybir.AxisListType.X

    nc = bacc.Bacc(
        "TRN2",
        target_bir_lowering=False,
        debug=debug,
        enable_asserts=False,
        num_devices=num_devices,
    )

    blob = nc.dram_tensor("blob", (R_IN, F), i8, kind="ExternalInput").ap()
    wblob = nc.dram_tensor("wblob", (R_WB, F), i8, kind="ExternalInput").ap()
    cmask = nc.dram_tensor("cmask", (C, C), f16, kind="ExternalInput").ap()
    bdmask = nc.dram_tensor("bdmask", (CH, W17), f32, kind="ExternalInput").ap()
    hmask = nc.dram_tensor("hmask", (CH, HEADS), f32, kind="ExternalInput").ap()
    yout = nc.dram_tensor("yout", (R_OUT, F), i8, kind="ExternalOutput").ap()

    xsc = blob[R_X:R_X + R_SC, :].bitcast(f16).rearrange("(a b) c -> a (b c)", b=2)
    o = 0
    wq = wblob[o:o + R_W, :].bitcast(f16).rearrange("(a b) c -> a (b c)", b=2)
    o += R_W
    wk = wblob[o:o + R_W, :].bitcast(f16).rearrange("(a b) c -> a (b c)", b=2)
    o += R_W
    wv = wblob[o:o + R_W, :].bitcast(f16).rearrange("(a b) c -> a (b c)", b=2)
    o += R_W
    bqkv = wblob[o:o + R_B, :].bitcast(f16).rearrange("(a b) c -> a (b c)", b=2)

    x3 = blob[0:R_X, :].rearrange("(t n) f -> t n f", n=NS)
    y3 = yout[0:R_X, :].rearrange("(t n) f -> t n f", n=NS)
    osc_out = yout[R_X:R_OUT, :].bitcast(f16).rearrange("(a b) c -> a (b c)", b=2)

    with tile.TileContext(nc) as tc, ExitStack() as ctx:
        cpool = ctx.enter_context(tc.tile_pool(name="consts", bufs=1))
        wq_sb = cpool.tile([F, CH], f16, tag="wq")
        wk_sb = cpool.tile([F, CH], f16, tag="wk")
        wv_sb = cpool.tile([F, CH], f16, tag="wv")
        nc.scalar.dma_start(wq_sb[:], wq)
        nc.scalar.dma_start(wk_sb[:], wk)
        nc.scalar.dma_start(wv_sb[:], wv)
        bq_sb = cpool.tile([1, CH], f16, tag="bq")
        bk_sb = cpool.tile([1, CH], f16, tag="bk")
        bv_sb = cpool.tile([1, CH], f16, tag="bv")
        nc.scalar.dma_start(bq_sb[:], bqkv[0:1, :])
        nc.scalar.dma_start(bk_sb[:], bqkv[1:2, :])
        nc.scalar.dma_start(bv_sb[:], bqkv[2:3, :])
        ones_sb = cpool.tile([1, C], f16, tag="ones")
        nc.vector.memset(ones_sb[:], 1.0)
        cm_sb = cpool.tile([C, C], f16, tag="cmask")
        nc.scalar.dma_start(cm_sb[:], cmask)
        bd_sb = cpool.tile([CH, W17], f32, tag="bdmask")
        nc.scalar.dma_start(bd_sb[:], bdmask)
        hm_sb = cpool.tile([CH, HEADS], f32, tag="hmask")
        nc.scalar.dma_start(hm_sb[:], hmask)
        xsc16 = cpool.tile([C, NCOL], f16, tag="xsc16")
        nc.scalar.dma_start(xsc16[:], xsc)
        xsc_sb = cpool.tile([C, NCOL], f32, tag="xsc")
        nc.vector.tensor_copy(xsc_sb[:], xsc16[:])
        osc_sb = cpool.tile([C, NCOL], f16, tag="osc")

        xpool = ctx.enter_context(tc.tile_pool(name="x", bufs=3))
        phipool = ctx.enter_context(tc.tile_pool(name="phi", bufs=3))
        spool = ctx.enter_context(tc.tile_pool(name="sacc", bufs=1))
        tpool = ctx.enter_context(tc.tile_pool(name="tmp", bufs=2))
        opool = ctx.enter_context(tc.tile_pool(name="out", bufs=3))
        ps_proj = ctx.enter_context(tc.tile_pool(name="psproj", bufs=4, space="PSUM"))
        ps_at = ctx.enter_context(tc.tile_pool(name="psat", bufs=1, space="PSUM"))
        ps_acc = ctx.enter_context(tc.tile_pool(name="psacc", bufs=1, space="PSUM"))
        ps_inta = ctx.enter_context(tc.tile_pool(name="psinta", bufs=1, space="PSUM"))
        ps_g = ctx.enter_context(tc.tile_pool(name="psg", bufs=1, space="PSUM"))

        def phi(dst, ps):
            # phi(x) = elu(x) + 1 = relu(x) + exp(min(x, 0))
            shape = [ps.shape[0], ps.shape[1]]
            a = tpool.tile(shape, f32, tag="phia")
            b = tpool.tile(shape, f32, tag="phib")
            nc.scalar.activation(a[:], ps[:], Relu)
            nc.vector.tensor_scalar_min(b[:], ps[:], 0.0)
            nc.scalar.activation(b[:], b[:], Exp)
            nc.vector.tensor_add(dst[:], a[:], b[:])

        for n in range(NS):
            S_acc = spool.tile([CH, W17], f32, tag="sacc")
            nc.vector.memset(S_acc[:], 0.0)
            for c in range(NC):
                col = n * NC + c
                # load int8 chunk [tok, F], dequant per-token, transpose to [F, tok]
                xi8 = xpool.tile([C, F], i8, tag="xi8")
                nc.scalar.dma_start(xi8[:], x3[c * C:(c + 1) * C, n, :])
                x16 = xpool.tile([C, F], f16, tag="x16")
                nc.vector.tensor_scalar_mul(x16[:], xi8[:], xsc_sb[:, col:col + 1])
                xT = xpool.tile([F, C], f16, tag="xT")
                nc.sync.dma_start(xT[:], x16[:], transpose=True)

                # projections (+ rank-1 bias add)
                qT_ps = ps_proj.tile([CH, C], f32, tag="proj")
                kT_ps = ps_proj.tile([CH, C], f32, tag="proj")
                kt_ps = ps_proj.tile([C, CH], f32, tag="proj")
                vt_ps = ps_proj.tile([C, CH], f32, tag="proj")
                nc.tensor.matmul(qT_ps[:], wq_sb[:], xT[:], start=True, stop=False)
                nc.tensor.matmul(qT_ps[:], bq_sb[:], ones_sb[:], start=False, stop=True)
                nc.tensor.matmul(kT_ps[:], wk_sb[:], xT[:], start=True, stop=False)
                nc.tensor.matmul(kT_ps[:], bk_sb[:], ones_sb[:], start=False, stop=True)
                nc.tensor.matmul(kt_ps[:], xT[:], wk_sb[:], start=True, stop=False)
                nc.tensor.matmul(kt_ps[:], ones_sb[:], bk_sb[:], start=False, stop=True)
                nc.tensor.matmul(vt_ps[:], xT[:], wv_sb[:], start=True, stop=False)
                nc.tensor.matmul(vt_ps[:], ones_sb[:], bv_sb[:], start=False, stop=True)

                q16 = phipool.tile([CH, C], f16, tag="q16")   # phi(q)^T [chan, tok]
                k16 = phipool.tile([CH, C], f16, tag="k16")   # phi(k)^T [chan, tok]
                kt16 = phipool.tile([C, CH], f16, tag="kt16")  # phi(k) [tok, chan]
                phi(q16, qT_ps)
                phi(k16, kT_ps)
                phi(kt16, kt_ps)

                # v_aug [tok, 136]: per head h cols h*17..h*17+15 = v_h, col h*17+16 = 1
                vaug = phipool.tile([C, W17], f16, tag="vaug")
                va = vaug[:].rearrange("p (h j) -> p h j", j=17)
                vs = vt_ps[:].rearrange("p (h j) -> p h j", j=16)
                nc.vector.tensor_copy(va[:, :, 0:16], vs)
                nc.vector.memset(va[:, :, 16:17], 1.0)

                # inter-chunk: acc[t, :] = phi(q)_t @ S_prev (block-diag interleaved)
                s16 = phipool.tile([CH, W17], f16, tag="s16")
                nc.vector.tensor_copy(s16[:], S_acc[:])
                acc_ps = ps_acc.tile([C, W17], f32, tag="acc")
                nc.tensor.matmul(acc_ps[:], q16[:], s16[:], start=True, stop=True)

                # intra-chunk per head: A^T = (k.head_mask)^T q (K=128, head-
                # masked k zeroes cross-head terms), mask causal, A_m^T.T@[v|1]
                inta_ps = ps_inta.tile([C, W17], f32, tag="inta")
                for h in range(HEADS):
                    kh = tpool.tile([CH, C], f16, tag="kh")
                    nc.vector.tensor_scalar_mul(kh[:], k16[:], hm_sb[:, h:h + 1])
                    at_ps = ps_at.tile([C, C], f32, tag="at")
                    nc.tensor.matmul(
                        at_ps[:], kh[:], q16[:], start=True, stop=True,
                    )
                    am = tpool.tile([C, C], f16, tag="am")
                    nc.vector.tensor_mul(am[:], at_ps[:], cm_sb[:])
                    nc.tensor.matmul(
                        inta_ps[:, h * 17:h * 17 + 17],
                        am[:],
                        vaug[:, h * 17:h * 17 + 17],
                        start=True, stop=True,
                    )

                # KV gram for this chunk + masked accumulate into S
                g_ps = ps_g.tile([CH, W17], f32, tag="g")
                nc.tensor.matmul(g_ps[:], kt16[:], vaug[:], start=True, stop=True)
                gt = tpool.tile([CH, W17], f32, tag="gt")
                nc.vector.tensor_mul(gt[:], g_ps[:], bd_sb[:])
                nc.vector.tensor_add(S_acc[:], S_acc[:], gt[:])

                # normalize: out = (num_inter + num_intra) / (den_i + den_x + eps)
                # DVE reads at most one PSUM operand: stage intra to SBUF first.
                inta_sb = tpool.tile([C, W17], f32, tag="intasb")
                nc.vector.tensor_copy(inta_sb[:], inta_ps[:])
                accv = acc_ps[:].rearrange("p (h j) -> p h j", j=17)
                intav = inta_sb[:].rearrange("p (h j) -> p h j", j=17)
                den = tpool.tile([C, HEADS], f32, tag="den")
                dv = den[:].rearrange("p (h j) -> p h j", j=1)
                nc.vector.scalar_tensor_tensor(
                    dv, accv[:, :, 16:17], EPS, intav[:, :, 16:17],
                    op0=AluOp.add, op1=AluOp.add,
                )
                rec = tpool.tile([C, HEADS], f32, tag="rec")
                nc.vector.reciprocal(rec[:], den[:])
                out_f = opool.tile([C, CH], f32, tag="outf")
                for h in range(HEADS):
                    nsum = tpool.tile([C, E], f32, tag="nsum")
                    nc.vector.tensor_add(
                        nsum[:],
                        acc_ps[:, h * 17:h * 17 + 16],
                        inta_sb[:, h * 17:h * 17 + 16],
                    )
                    nc.vector.tensor_scalar_mul(
                        out_f[:, h * 16:(h + 1) * 16],
                        nsum[:],
                        rec[:, h:h + 1],
                    )

                # int8 quantize per token: amax, scale out, store scale
                amax = tpool.tile([C, 1], f32, tag="amax")
                nc.vector.reduce_max(
                    amax[:], out_f[:], axis=AX, apply_absolute_value=True
                )
                nc.vector.tensor_scalar_max(amax[:], amax[:], AMAX_FLOOR)
                nc.vector.tensor_scalar_mul(
                    osc_sb[:, col:col + 1], amax[:], 1.0 / 127.0
                )
                r8 = tpool.tile([C, 1], f32, tag="r8")
                nc.vector.reciprocal(r8[:], amax[:])
                y8t = opool.tile([C, CH], i8, tag="y8t")
                nc.vector.tensor_scalar(
                    y8t[:], out_f[:], r8[:, 0:1], 127.0,
                    op0=AluOp.mult, op1=AluOp.mult,
                )
                nc.gpsimd.dma_start(y3[c * C:(c + 1) * C, n, :], y8t[:])

        nc.gpsimd.dma_start(osc_out, osc_sb[:])

    nc.compile()
    return nc


def _consts():
    cmask = np.triu(np.ones((C, C), np.float16))  # cmask[s,t] = 1 if s<=t
    bd = np.zeros((CH, W17), np.float32)
    for h in range(HEADS):
        bd[h * 16:(h + 1) * 16, h * 17:(h + 1) * 17] = 1.0
    hm = np.zeros((CH, HEADS), np.float32)
    for h in range(HEADS):
        hm[h * 16:(h + 1) * 16, h] = 1.0
    return cmask, bd, hm


def _pack_batch(blob, i, query_b, s, qscratch):
    """Quantize node-slice s of one batch into position i of an input blob."""
    base = i * R_IN
    n0 = s * NS
    xs = query_b[:, n0:n0 + NS, :]  # (L, NS, F) f32 view
    amax = np.maximum(xs.max(axis=2), -xs.min(axis=2))  # (L, NS)
    np.maximum(amax, 1e-12, out=amax)
    inv = np.float32(127.0) / amax
    q = np.multiply(xs, inv[:, :, None], out=qscratch)
    np.rint(q, out=q)
    xv = blob[base:base + R_X, :].reshape(L, NS, F)
    np.copyto(xv, q, casting="unsafe")
    # scales f16, layout [j, n*NC+c] from amax[t= c*C+j, n]
    sc = amax * np.float32(1.0 / 127.0)
    sct = sc.reshape(NC, C, NS).transpose(1, 2, 0).reshape(C, NCOL)
    scv = blob[base + R_X:base + R_IN, :].view(np.float16)
    scv.reshape(C, NCOL)[:] = sct


def _pack_weights(nd, Wq, bq_, Wk, bk_, Wv, bv_):
    """f16 weight+bias byte block, tiled for nd cores."""
    wbytes = np.empty((nd, R_WB, F), np.int8)
    w0 = wbytes[0]
    wv16 = w0[0:3 * R_W, :].view(np.float16).reshape(3, F, CH)
    wv16[0] = Wq
    wv16[1] = Wk
    wv16[2] = Wv
    bv16 = w0[3 * R_W:, :].view(np.float16).reshape(3, CH)
    bv16[0] = bq_
    bv16[1] = bk_
    bv16[2] = bv_
    wbytes[1:] = w0
    return wbytes.reshape(nd * R_WB, F)


def _make_io(devices):
    """Device I/O driver for len(devices) cores: dispatch / fetch+dequant."""
    import jax
    from jax.sharding import Mesh, NamedSharding, PartitionSpec

    try:
        from jax.experimental.shard_map import shard_map
    except ImportError:
        from jax.shard_map import shard_map

    import concourse.mybir as mybir
    from concourse.bass2jax import (
        _bass_exec_p,
        install_neuronx_cc_hook,
        partition_id_tensor,
    )

    install_neuronx_cc_hook()
    nd = len(devices)
    nc = build_nc(nd)

    partition_name = (
        nc.partition_id_tensor.name if nc.partition_id_tensor is not None else None
    )
    in_names: list[str] = []
    out_names: list[str] = []
    out_avals = []
    zero_outs = []
    for alloc in nc.m.functions[0].allocations:
        if not isinstance(alloc, mybir.MemoryLocationSet):
            continue
        name = alloc.memorylocations[0].name
        if alloc.kind == "ExternalInput":
            if name != partition_name:
                in_names.append(name)
        elif alloc.kind == "ExternalOutput":
            out_names.append(name)
            shape = tuple(alloc.tensor_shape)
            dtype = mybir.dt.np(alloc.dtype)
            out_avals.append(jax.core.ShapedArray(shape, dtype))
            zero_outs.append(np.zeros((nd * shape[0], *shape[1:]), dtype))
    n_params = len(in_names)
    all_in_names = in_names + out_names
    if partition_name is not None:
        all_in_names = all_in_names + [partition_name]

    def _body(*args):
        operands = list(args)
        if partition_name is not None:
            operands.append(partition_id_tensor())
        outs = _bass_exec_p.bind(
            *operands,
            out_avals=tuple(out_avals),
            in_names=tuple(all_in_names),
            out_names=tuple(out_names),
            lowering_input_output_aliases=(),
            sim_require_finite=True,
            sim_require_nnan=True,
            nc=nc,
        )
        return tuple(outs)

    mesh = Mesh(np.asarray(devices), ("core",))
    spec = NamedSharding(mesh, PartitionSpec("core"))
    nin = n_params + len(out_names)
    sharded = jax.jit(
        shard_map(
            _body,
            mesh=mesh,
            in_specs=(PartitionSpec("core"),) * nin,
            out_specs=(PartitionSpec("core"),) * len(out_names),
            check_rep=False,
        ),
        keep_unused=True,
    )

    cmask, bd, hm = _consts()
    persist = {
        "cmask": jax.device_put(np.tile(cmask, (nd, 1)), spec),
        "bdmask": jax.device_put(np.tile(bd, (nd, 1)), spec),
        "hmask": jax.device_put(np.tile(hm, (nd, 1)), spec),
    }
    zeros_dev = [jax.device_put(z, spec) for z in zero_outs]
    yi = {nm: i for i, nm in enumerate(out_names)}["yout"]

    assert in_names[0] == "blob" and in_names[1] == "wblob", in_names
    assert all(nm in persist for nm in in_names[2:]), in_names
    persist_args = tuple(persist[nm] for nm in in_names[2:])

    def put_weights(wbytes):
        return jax.device_put(wbytes, spec)

    def dispatch(blob, wdev):
        out = sharded(blob, wdev, *persist_args, *zeros_dev)
        y = out[yi]
        y.copy_to_host_async()
        return y

    def fetch_dequant(y, s, batches, res):
        n0 = s * NS
        for sh in y.addressable_shards:
            i = sh.index[0].start // R_OUT if sh.index else 0
            b = batches[i]
            yh = np.asarray(sh.data)
            yb = yh[0:R_X, :].reshape(L, NS, CH)
            oscb = yh[R_X:R_OUT, :].view(np.float16)
            sf = (
                oscb.reshape(C, NS, NC)
                .transpose(2, 0, 1)
                .reshape(L, NS)
                .astype(np.float32)
            )
            np.multiply(yb, sf[:, :, None], out=res[b, :, n0:n0 + NS, :])

    return put_weights, dispatch, fetch_dequant


# ── shared-memory layout for the multi-process path ──────────────────────
_Q_BYTES = NWORK * S * ND * R_IN * F          # packed int8 input blobs
_R_BYTES = 8 * L * NSEQ * CH * 4              # f32 result
_W_BYTES = NW_F32 * 4                         # f32 weights+biases
# control: [go_(w,s) x NWORK*S, done_w x NWORK, ready_w x NWORK, quit]
_CTL_WORDS = NWORK * S + 2 * NWORK + 1


def _shm_views(shm_b, shm_r, shm_w, shm_c):
    bv = np.ndarray((NWORK, S, ND * R_IN, F), np.int8, buffer=shm_b.buf)
    rv = np.ndarray((8, L, NSEQ, CH), np.float32, buffer=shm_r.buf)
    wv = np.ndarray((NW_F32,), np.float32, buffer=shm_w.buf)
    cv = np.ndarray((_CTL_WORDS,), np.int64, buffer=shm_c.buf)
    return bv, rv, wv, cv


def _worker_main(w, tag):
    """Worker subprocess: thin I/O driver for cores [w*ND, (w+1)*ND)."""
    from multiprocessing import shared_memory

    shms = [
        shared_memory.SharedMemory(name=f"{tag}_{x}") for x in ("b", "r", "w", "c")
    ]
    bv, rv, wv, ctl = _shm_views(*shms)

    import jax

    devices = jax.devices()[w * ND:(w + 1) * ND]
    batches = list(range(w * ND, (w + 1) * ND))
    put_weights, dispatch, fetch_dequant = _make_io(devices)

    # warm the transfer path with realistic random data (own shm region)
    rng = np.random.default_rng(w)
    qscratch = np.empty((L, NS, F), np.float32)
    dq = rng.standard_normal((L, NSEQ, F)).astype(np.float32)
    for s in range(S):
        for i in range(ND):
            _pack_batch(bv[w, s], i, dq, s, qscratch)
    dwb = _pack_weights(
        ND,
        rng.standard_normal((F, CH)).astype(np.float32) * np.float32(0.09),
        np.zeros(CH, np.float32),
        rng.standard_normal((F, CH)).astype(np.float32) * np.float32(0.09),
        np.zeros(CH, np.float32),
        rng.standard_normal((F, CH)).astype(np.float32) * np.float32(0.09),
        np.zeros(CH, np.float32),
    )
    for _ in range(2):
        wdev = put_weights(dwb)
        ys = [dispatch(bv[w, s], wdev) for s in range(S)]
        for s in range(S):
            fetch_dequant(ys[s], s, batches, rv)

    import gc

    gc.collect()
    gc.freeze()

    goff = w * S
    doff = NWORK * S
    roff = NWORK * S + NWORK
    qoff = NWORK * S + 2 * NWORK
    ctl[roff + w] = 1  # ready
    seen = 0
    parent = os.getppid()
    while True:
        g = int(ctl[goff])
        if g == seen:
            if ctl[qoff] or os.getppid() != parent:
                break
            time.sleep(0.0005)
            continue
        seen = g
        W = wv
        wb = _pack_weights(
            ND,
            W[0:F * CH].reshape(F, CH),
            W[3 * F * CH:3 * F * CH + CH],
            W[F * CH:2 * F * CH].reshape(F, CH),
            W[3 * F * CH + CH:3 * F * CH + 2 * CH],
            W[2 * F * CH:3 * F * CH].reshape(F, CH),
            W[3 * F * CH + 2 * CH:3 * F * CH + 3 * CH],
        )
        wdev = put_weights(wb)
        ys = []
        for s in range(S):
            while int(ctl[goff + s]) != g:
                time.sleep(0.0002)
            ys.append(dispatch(bv[w, s], wdev))
        for s in range(S):
            fetch_dequant(ys[s], s, batches, rv)
        ctl[doff + w] = g  # done


class _MPRunner:
    def __init__(self):
        from multiprocessing import shared_memory

        tag = f"clk{os.getpid()}"
        self.tag = tag
        sizes = {"b": _Q_BYTES, "r": _R_BYTES, "w": _W_BYTES, "c": _CTL_WORDS * 8}
        self.shms = [
            shared_memory.SharedMemory(name=f"{tag}_{x}", create=True, size=sz)
            for x, sz in sizes.items()
        ]
        self.bv, self.rv, self.wv, self.ctl = _shm_views(*self.shms)
        self.ctl[:] = 0
        self.seq = 0
        self.qscratch = np.empty((L, NS, F), np.float32)
        self.procs = []

        import subprocess

        here = os.path.dirname(os.path.abspath(__file__))
        code = (
            "import sys; sys.path.insert(0, %r); "
            "import kernel; kernel._worker_main(int(sys.argv[1]), %r)"
        ) % (here, tag)

        def spawn(w):
            return subprocess.Popen(
                [sys.executable, "-c", code, str(w)],
                stdout=subprocess.DEVNULL,
                stderr=subprocess.DEVNULL,
                cwd=here,
            )

        # worker 0 first so it fills the neuronxcc NEFF cache; the rest
        # start once it is ready and hit the warm cache
        self.procs.append(spawn(0))
        self._wait_ready([0], timeout=2400)
        for w in range(1, NWORK):
            self.procs.append(spawn(w))
        self._wait_ready(range(1, NWORK), timeout=2400)

        import atexit

        atexit.register(self._cleanup)

    def _wait_ready(self, ws, timeout):
        roff = NWORK * S + NWORK
        t0 = time.time()
        for w in ws:
            while not self.ctl[roff + w]:
                if self.procs[min(w, len(self.procs) - 1)].poll() is not None:
                    raise RuntimeError(f"worker {w} died during startup")
                if time.time() - t0 > timeout:
                    raise RuntimeError(f"worker {w} startup timeout")
                time.sleep(0.01)

    def _cleanup(self):
        try:
            self.ctl[NWORK * S + 2 * NWORK] = 1
            for p in self.procs:
                try:
                    p.wait(timeout=2)
                except Exception:
                    p.kill()
        except Exception:
            pass
        for s in self.shms:
            try:
                s.close()
                s.unlink()
            except Exception:
                pass

    def __call__(self, query, Wq, bq_, Wk, bk_, Wv, bv_):
        W = self.wv
        W[0:F * CH] = np.asarray(Wq, np.float32).reshape(-1)
        W[F * CH:2 * F * CH] = np.asarray(Wk, np.float32).reshape(-1)
        W[2 * F * CH:3 * F * CH] = np.asarray(Wv, np.float32).reshape(-1)
        boff = 3 * F * CH
        W[boff:boff + CH] = bq_
        W[boff + CH:boff + 2 * CH] = bk_
        W[boff + 2 * CH:boff + 3 * CH] = bv_

        self.seq += 1
        ctl = self.ctl
        # breadth-first packing: every worker gets its slice-0 blob (and its
        # go signal) before any slice-1 work, so all four tunnel connections
        # start uploading as early as possible
        for s in range(S):
            for w in range(NWORK):
                blob = self.bv[w, s]
                for i in range(ND):
                    _pack_batch(blob, i, query[w * ND + i], s, self.qscratch)
                ctl[w * S + s] = self.seq
        doff = NWORK * S
        deadline = time.time() + 120
        for w in range(NWORK):
            while int(ctl[doff + w]) != self.seq:
                if time.time() > deadline:
                    raise RuntimeError(f"worker {w} call timeout")
                time.sleep(0.0002)
        return self.rv


_RUNNER = None


def _make_single_process_runner():
    import jax

    put_weights, dispatch, fetch_dequant = _make_io(jax.devices()[:8])
    batches = list(range(8))
    blobs = [np.empty((8 * R_IN, F), np.int8) for _ in range(S)]
    qscratch = np.empty((L, NS, F), np.float32)
    res_buf = np.empty((8, L, NSEQ, CH), np.float32)

    def call(query, Wq, bq_, Wk, bk_, Wv, bv_):
        query = np.asarray(query, np.float32)
        wdev = put_weights(_pack_weights(8, Wq, bq_, Wk, bk_, Wv, bv_))
        ys = []
        for s in range(S):
            for i in range(8):
                _pack_batch(blobs[s], i, query[i], s, qscratch)
            ys.append(dispatch(blobs[s], wdev))
        for s in range(S):
            fetch_dequant(ys[s], s, batches, res_buf)
        return res_buf

    rng = np.random.default_rng(0)
    dq = rng.standard_normal((8, L, NSEQ, F)).astype(np.float32)
    dw = rng.standard_normal((F, CH)).astype(np.float32) * np.float32(0.09)
    db = np.zeros((CH,), np.float32)
    for _ in range(2):
        call(dq, dw, db, dw, db, dw, db)

    import gc

    gc.collect()
    gc.freeze()

    return call


def kernel(query, Wq, bq, Wk, bk, Wv, bv):
    global _RUNNER
    if _RUNNER is None:
        try:
            _RUNNER = _MPRunner()
        except Exception:
            _RUNNER = _make_single_process_runner()
    return _RUNNER(np.asarray(query, np.float32), Wq, bq, Wk, bk, Wv, bv)


# revision 28
# speedup vs baseline: 1.0147x; 1.0147x over previous
"""Causal linear attention (fast_transformers style) on 8 Trainium2 cores.

query (8, 512, 64, 128) f32. Data-parallel: one batch element per core.
Per (batch, node) sequence of L=512 tokens: project q/k/v with 128x128
weights, phi(x)=elu(x)+1, causal linear attention via chunked scan
(C=128 intra-chunk masked matmul + inter-chunk running KV state).

Wire strategy: the axon tunnel is half-duplex at ~40-50 MiB/s and is the
whole cost, so ship int8 with per-token scales both directions. All of a
slice-call's per-core inputs (int8 x, f16 scales) are packed into ONE
int8 blob (f16 regions read on device via bitcast APs) so each direction
is a single large transfer per call; the f16 weights/biases go up once
per kernel() call as a separate device-resident buffer, async, hidden
under the first quantization. The node axis is split into S=4 slices
pipelined as async calls so host quant/dequant and device exec hide
under the serialized wire time. Host buffers are preallocated and reused
(the box has ONE cpu - allocator churn showed up as 20% call-time
drift). Matmuls run fp16 with fp32 PSUM accumulation; mask constants
and output zero-buffers stay device-resident.
"""

import numpy as np

HEADS = 8
E = 16
EPS = 1e-6
L = 512
NSEQ = 64
F = 128
CH = HEADS * E  # 128 output channels
C = 128         # time chunk
NC = L // C
W17 = 17 * HEADS  # 136: per-head [num(16) | den(1)] interleaved width
AMAX_FLOOR = 1e-4

S = 4                  # node-axis slices pipelined per kernel() call
NS = NSEQ // S         # nodes per core per slice-call
NCOL = NS * NC         # scale columns per slice: one per (n, c) tile

# int8 blob row layout (rows x 128 bytes), input side
R_X = L * NS                         # xq int8 rows, row = t*NS + n
R_SC = C * NCOL * 2 // F             # f16 scales region rows
R_W = (F * CH * 2) // F              # one f16 weight matrix region rows
R_B = (3 * CH * 2 + F - 1) // F      # three f16 biases region rows
R_IN = R_X + R_SC
R_WB = 3 * R_W + R_B                 # separate weight blob, uploaded once/call
R_OUT = R_X + R_SC                   # y8 rows + f16 out-scales rows


def build_nc(num_devices, debug=False):
    """Build the per-core Bass module for one slice-call (NS node seqs)."""
    from contextlib import ExitStack

    import concourse.bacc as bacc
    import concourse.mybir as mybir
    import concourse.tile as tile

    i8 = mybir.dt.int8
    f16 = mybir.dt.float16
    f32 = mybir.dt.float32
    Relu = mybir.ActivationFunctionType.Relu
    Exp = mybir.ActivationFunctionType.Exp
    AluOp = mybir.AluOpType
    AX = mybir.AxisListType.X

    nc = bacc.Bacc(
        "TRN2",
        target_bir_lowering=False,
        debug=debug,
        enable_asserts=False,
        num_devices=num_devices,
    )

    blob = nc.dram_tensor("blob", (R_IN, F), i8, kind="ExternalInput").ap()
    wblob = nc.dram_tensor("wblob", (R_WB, F), i8, kind="ExternalInput").ap()
    cmask = nc.dram_tensor("cmask", (C, C), f16, kind="ExternalInput").ap()
    bdmask = nc.dram_tensor("bdmask", (CH, W17), f32, kind="ExternalInput").ap()
    hmask = nc.dram_tensor("hmask", (CH, HEADS), f32, kind="ExternalInput").ap()
    yout = nc.dram_tensor("yout", (R_OUT, F), i8, kind="ExternalOutput").ap()

    xsc = blob[R_X:R_X + R_SC, :].bitcast(f16).rearrange("(a b) c -> a (b c)", b=2)
    o = 0
    wq = wblob[o:o + R_W, :].bitcast(f16).rearrange("(a b) c -> a (b c)", b=2)
    o += R_W
    wk = wblob[o:o + R_W, :].bitcast(f16).rearrange("(a b) c -> a (b c)", b=2)
    o += R_W
    wv = wblob[o:o + R_W, :].bitcast(f16).rearrange("(a b) c -> a (b c)", b=2)
    o += R_W
    bqkv = wblob[o:o + R_B, :].bitcast(f16).rearrange("(a b) c -> a (b c)", b=2)

    x3 = blob[0:R_X, :].rearrange("(t n) f -> t n f", n=NS)
    y3 = yout[0:R_X, :].rearrange("(t n) f -> t n f", n=NS)
    osc_out = yout[R_X:R_OUT, :].bitcast(f16).rearrange("(a b) c -> a (b c)", b=2)

    with tile.TileContext(nc) as tc, ExitStack() as ctx:
        cpool = ctx.enter_context(tc.tile_pool(name="consts", bufs=1))
        wq_sb = cpool.tile([F, CH], f16, tag="wq")
        wk_sb = cpool.tile([F, CH], f16, tag="wk")
        wv_sb = cpool.tile([F, CH], f16, tag="wv")
        nc.scalar.dma_start(wq_sb[:], wq)
        nc.scalar.dma_start(wk_sb[:], wk)
        nc.scalar.dma_start(wv_sb[:], wv)
        bq_sb = cpool.tile([1, CH], f16, tag="bq")
        bk_sb = cpool.tile([1, CH], f16, tag="bk")
        bv_sb = cpool.tile([1, CH], f16, tag="bv")
        nc.scalar.dma_start(bq_sb[:], bqkv[0:1, :])
        nc.scalar.dma_start(bk_sb[:], bqkv[1:2, :])
        nc.scalar.dma_start(bv_sb[:], bqkv[2:3, :])
        ones_sb = cpool.tile([1, C], f16, tag="ones")
        nc.vector.memset(ones_sb[:], 1.0)
        cm_sb = cpool.tile([C, C], f16, tag="cmask")
        nc.scalar.dma_start(cm_sb[:], cmask)
        bd_sb = cpool.tile([CH, W17], f32, tag="bdmask")
        nc.scalar.dma_start(bd_sb[:], bdmask)
        hm_sb = cpool.tile([CH, HEADS], f32, tag="hmask")
        nc.scalar.dma_start(hm_sb[:], hmask)
        xsc16 = cpool.tile([C, NCOL], f16, tag="xsc16")
        nc.scalar.dma_start(xsc16[:], xsc)
        xsc_sb = cpool.tile([C, NCOL], f32, tag="xsc")
        nc.vector.tensor_copy(xsc_sb[:], xsc16[:])
        osc_sb = cpool.tile([C, NCOL], f16, tag="osc")

        xpool = ctx.enter_context(tc.tile_pool(name="x", bufs=3))
        phipool = ctx.enter_context(tc.tile_pool(name="phi", bufs=3))
        spool = ctx.enter_context(tc.tile_pool(name="sacc", bufs=1))
        tpool = ctx.enter_context(tc.tile_pool(name="tmp", bufs=2))
        opool = ctx.enter_context(tc.tile_pool(name="out", bufs=3))
        ps_proj = ctx.enter_context(tc.tile_pool(name="psproj", bufs=4, space="PSUM"))
        ps_at = ctx.enter_context(tc.tile_pool(name="psat", bufs=1, space="PSUM"))
        ps_acc = ctx.enter_context(tc.tile_pool(name="psacc", bufs=1, space="PSUM"))
        ps_inta = ctx.enter_context(tc.tile_pool(name="psinta", bufs=1, space="PSUM"))
        ps_g = ctx.enter_context(tc.tile_pool(name="psg", bufs=1, space="PSUM"))

        def phi(dst, ps):
            # phi(x) = elu(x) + 1 = relu(x) + exp(min(x, 0))
            shape = [ps.shape[0], ps.shape[1]]
            a = tpool.tile(shape, f32, tag="phia")
            b = tpool.tile(shape, f32, tag="phib")
            nc.scalar.activation(a[:], ps[:], Relu)
            nc.vector.tensor_scalar_min(b[:], ps[:], 0.0)
            nc.scalar.activation(b[:], b[:], Exp)
            nc.vector.tensor_add(dst[:], a[:], b[:])

        for n in range(NS):
            S_acc = spool.tile([CH, W17], f32, tag="sacc")
            nc.vector.memset(S_acc[:], 0.0)
            for c in range(NC):
                col = n * NC + c
                # load int8 chunk [tok, F], dequant per-token, transpose to [F, tok]
                xi8 = xpool.tile([C, F], i8, tag="xi8")
                nc.scalar.dma_start(xi8[:], x3[c * C:(c + 1) * C, n, :])
                x16 = xpool.tile([C, F], f16, tag="x16")
                nc.vector.tensor_scalar_mul(x16[:], xi8[:], xsc_sb[:, col:col + 1])
                xT = xpool.tile([F, C], f16, tag="xT")
                nc.sync.dma_start(xT[:], x16[:], transpose=True)

                # projections (+ rank-1 bias add)
                qT_ps = ps_proj.tile([CH, C], f32, tag="proj")
                kT_ps = ps_proj.tile([CH, C], f32, tag="proj")
                kt_ps = ps_proj.tile([C, CH], f32, tag="proj")
                vt_ps = ps_proj.tile([C, CH], f32, tag="proj")
                nc.tensor.matmul(qT_ps[:], wq_sb[:], xT[:], start=True, stop=False)
                nc.tensor.matmul(qT_ps[:], bq_sb[:], ones_sb[:], start=False, stop=True)
                nc.tensor.matmul(kT_ps[:], wk_sb[:], xT[:], start=True, stop=False)
                nc.tensor.matmul(kT_ps[:], bk_sb[:], ones_sb[:], start=False, stop=True)
                nc.tensor.matmul(kt_ps[:], xT[:], wk_sb[:], start=True, stop=False)
                nc.tensor.matmul(kt_ps[:], ones_sb[:], bk_sb[:], start=False, stop=True)
                nc.tensor.matmul(vt_ps[:], xT[:], wv_sb[:], start=True, stop=False)
                nc.tensor.matmul(vt_ps[:], ones_sb[:], bv_sb[:], start=False, stop=True)

                q16 = phipool.tile([CH, C], f16, tag="q16")   # phi(q)^T [chan, tok]
                k16 = phipool.tile([CH, C], f16, tag="k16")   # phi(k)^T [chan, tok]
                kt16 = phipool.tile([C, CH], f16, tag="kt16")  # phi(k) [tok, chan]
                phi(q16, qT_ps)
                phi(k16, kT_ps)
                phi(kt16, kt_ps)

                # v_aug [tok, 136]: per head h cols h*17..h*17+15 = v_h, col h*17+16 = 1
                vaug = phipool.tile([C, W17], f16, tag="vaug")
                va = vaug[:].rearrange("p (h j) -> p h j", j=17)
                vs = vt_ps[:].rearrange("p (h j) -> p h j", j=16)
                nc.vector.tensor_copy(va[:, :, 0:16], vs)
                nc.vector.memset(va[:, :, 16:17], 1.0)

                # inter-chunk: acc[t, :] = phi(q)_t @ S_prev (block-diag interleaved)
                s16 = phipool.tile([CH, W17], f16, tag="s16")
                nc.vector.tensor_copy(s16[:], S_acc[:])
                acc_ps = ps_acc.tile([C, W17], f32, tag="acc")
                nc.tensor.matmul(acc_ps[:], q16[:], s16[:], start=True, stop=True)

                # intra-chunk per head: A^T = (k.head_mask)^T q (K=128, head-
                # masked k zeroes cross-head terms), mask causal, A_m^T.T@[v|1]
                inta_ps = ps_inta.tile([C, W17], f32, tag="inta")
                for h in range(HEADS):
                    kh = tpool.tile([CH, C], f16, tag="kh")
                    nc.vector.tensor_scalar_mul(kh[:], k16[:], hm_sb[:, h:h + 1])
                    at_ps = ps_at.tile([C, C], f32, tag="at")
                    nc.tensor.matmul(
                        at_ps[:], kh[:], q16[:], start=True, stop=True,
                    )
                    am = tpool.tile([C, C], f16, tag="am")
                    nc.vector.tensor_mul(am[:], at_ps[:], cm_sb[:])
                    nc.tensor.matmul(
                        inta_ps[:, h * 17:h * 17 + 17],
                        am[:],
                        vaug[:, h * 17:h * 17 + 17],
                        start=True, stop=True,
                    )

                # KV gram for this chunk + masked accumulate into S
                g_ps = ps_g.tile([CH, W17], f32, tag="g")
                nc.tensor.matmul(g_ps[:], kt16[:], vaug[:], start=True, stop=True)
                gt = tpool.tile([CH, W17], f32, tag="gt")
                nc.vector.tensor_mul(gt[:], g_ps[:], bd_sb[:])
                nc.vector.tensor_add(S_acc[:], S_acc[:], gt[:])

                # normalize: out = (num_inter + num_intra) / (den_i + den_x + eps)
                # DVE reads at most one PSUM operand: stage intra to SBUF first.
                inta_sb = tpool.tile([C, W17], f32, tag="intasb")
                nc.vector.tensor_copy(inta_sb[:], inta_ps[:])
                accv = acc_ps[:].rearrange("p (h j) -> p h j", j=17)
                intav = inta_sb[:].rearrange("p (h j) -> p h j", j=17)
                den = tpool.tile([C, HEADS], f32, tag="den")
                dv = den[:].rearrange("p (h j) -> p h j", j=1)
                nc.vector.scalar_tensor_tensor(
                    dv, accv[:, :, 16:17], EPS, intav[:, :, 16:17],
                    op0=AluOp.add, op1=AluOp.add,
                )
                rec = tpool.tile([C, HEADS], f32, tag="rec")
                nc.vector.reciprocal(rec[:], den[:])
                out_f = opool.tile([C, CH], f32, tag="outf")
                for h in range(HEADS):
                    nsum = tpool.tile([C, E], f32, tag="nsum")
                    nc.vector.tensor_add(
                        nsum[:],
                        acc_ps[:, h * 17:h * 17 + 16],
                        inta_sb[:, h * 17:h * 17 + 16],
                    )
                    nc.vector.tensor_scalar_mul(
                        out_f[:, h * 16:(h + 1) * 16],
                        nsum[:],
                        rec[:, h:h + 1],
                    )

                # int8 quantize per token: amax, scale out, store scale
                amax = tpool.tile([C, 1], f32, tag="amax")
                nc.vector.reduce_max(
                    amax[:], out_f[:], axis=AX, apply_absolute_value=True
                )
                nc.vector.tensor_scalar_max(amax[:], amax[:], AMAX_FLOOR)
                nc.vector.tensor_scalar_mul(
                    osc_sb[:, col:col + 1], amax[:], 1.0 / 127.0
                )
                r8 = tpool.tile([C, 1], f32, tag="r8")
                nc.vector.reciprocal(r8[:], amax[:])
                y8t = opool.tile([C, CH], i8, tag="y8t")
                nc.vector.tensor_scalar(
                    y8t[:], out_f[:], r8[:, 0:1], 127.0,
                    op0=AluOp.mult, op1=AluOp.mult,
                )
                nc.gpsimd.dma_start(y3[c * C:(c + 1) * C, n, :], y8t[:])

        nc.gpsimd.dma_start(osc_out, osc_sb[:])

    nc.compile()
    return nc


def _consts():
    cmask = np.triu(np.ones((C, C), np.float16))  # cmask[s,t] = 1 if s<=t
    bd = np.zeros((CH, W17), np.float32)
    for h in range(HEADS):
        bd[h * 16:(h + 1) * 16, h * 17:(h + 1) * 17] = 1.0
    hm = np.zeros((CH, HEADS), np.float32)
    for h in range(HEADS):
        hm[h * 16:(h + 1) * 16, h] = 1.0
    return cmask, bd, hm


def _pack_batch(blob, i, query_b, s, qscratch):
    """Quantize node-slice s of one batch into position i of an input blob."""
    base = i * R_IN
    n0 = s * NS
    xs = query_b[:, n0:n0 + NS, :]  # (L, NS, F) f32 view
    amax = np.maximum(xs.max(axis=2), -xs.min(axis=2))  # (L, NS)
    np.maximum(amax, 1e-12, out=amax)
    inv = np.float32(127.0) / amax
    q = np.multiply(xs, inv[:, :, None], out=qscratch)
    np.rint(q, out=q)
    xv = blob[base:base + R_X, :].reshape(L, NS, F)
    np.copyto(xv, q, casting="unsafe")
    # scales f16, layout [j, n*NC+c] from amax[t= c*C+j, n]
    sc = amax * np.float32(1.0 / 127.0)
    sct = sc.reshape(NC, C, NS).transpose(1, 2, 0).reshape(C, NCOL)
    scv = blob[base + R_X:base + R_IN, :].view(np.float16)
    scv.reshape(C, NCOL)[:] = sct


def _pack_weights(nd, Wq, bq_, Wk, bk_, Wv, bv_):
    """f16 weight+bias byte block, tiled for nd cores."""
    wbytes = np.empty((nd, R_WB, F), np.int8)
    w0 = wbytes[0]
    wv16 = w0[0:3 * R_W, :].view(np.float16).reshape(3, F, CH)
    wv16[0] = Wq
    wv16[1] = Wk
    wv16[2] = Wv
    bv16 = w0[3 * R_W:, :].view(np.float16).reshape(3, CH)
    bv16[0] = bq_
    bv16[1] = bk_
    bv16[2] = bv_
    wbytes[1:] = w0
    return wbytes.reshape(nd * R_WB, F)


def _make_io(devices):
    """Device I/O driver for len(devices) cores: dispatch / fetch+dequant."""
    import jax
    from jax.sharding import Mesh, NamedSharding, PartitionSpec

    try:
        from jax.experimental.shard_map import shard_map
    except ImportError:
        from jax.shard_map import shard_map

    import concourse.mybir as mybir
    from concourse.bass2jax import (
        _bass_exec_p,
        install_neuronx_cc_hook,
        partition_id_tensor,
    )

    install_neuronx_cc_hook()
    nd = len(devices)
    nc = build_nc(nd)

    partition_name = (
        nc.partition_id_tensor.name if nc.partition_id_tensor is not None else None
    )
    in_names: list[str] = []
    out_names: list[str] = []
    out_avals = []
    zero_outs = []
    for alloc in nc.m.functions[0].allocations:
        if not isinstance(alloc, mybir.MemoryLocationSet):
            continue
        name = alloc.memorylocations[0].name
        if alloc.kind == "ExternalInput":
            if name != partition_name:
                in_names.append(name)
        elif alloc.kind == "ExternalOutput":
            out_names.append(name)
            shape = tuple(alloc.tensor_shape)
            dtype = mybir.dt.np(alloc.dtype)
            out_avals.append(jax.core.ShapedArray(shape, dtype))
            zero_outs.append(np.zeros((nd * shape[0], *shape[1:]), dtype))
    n_params = len(in_names)
    all_in_names = in_names + out_names
    if partition_name is not None:
        all_in_names = all_in_names + [partition_name]

    def _body(*args):
        operands = list(args)
        if partition_name is not None:
            operands.append(partition_id_tensor())
        outs = _bass_exec_p.bind(
            *operands,
            out_avals=tuple(out_avals),
            in_names=tuple(all_in_names),
            out_names=tuple(out_names),
            lowering_input_output_aliases=(),
            sim_require_finite=True,
            sim_require_nnan=True,
            nc=nc,
        )
        return tuple(outs)

    mesh = Mesh(np.asarray(devices), ("core",))
    spec = NamedSharding(mesh, PartitionSpec("core"))
    nin = n_params + len(out_names)
    sharded = jax.jit(
        shard_map(
            _body,
            mesh=mesh,
            in_specs=(PartitionSpec("core"),) * nin,
            out_specs=(PartitionSpec("core"),) * len(out_names),
            check_rep=False,
        ),
        keep_unused=True,
    )

    cmask, bd, hm = _consts()
    persist = {
        "cmask": jax.device_put(np.tile(cmask, (nd, 1)), spec),
        "bdmask": jax.device_put(np.tile(bd, (nd, 1)), spec),
        "hmask": jax.device_put(np.tile(hm, (nd, 1)), spec),
    }
    zeros_dev = [jax.device_put(z, spec) for z in zero_outs]
    yi = {nm: i for i, nm in enumerate(out_names)}["yout"]

    assert in_names[0] == "blob" and in_names[1] == "wblob", in_names
    assert all(nm in persist for nm in in_names[2:]), in_names
    persist_args = tuple(persist[nm] for nm in in_names[2:])

    def put_weights(wbytes):
        return jax.device_put(wbytes, spec)

    def dispatch(blob, wdev):
        out = sharded(blob, wdev, *persist_args, *zeros_dev)
        y = out[yi]
        y.copy_to_host_async()
        return y

    def fetch_dequant(y, s, batches, res):
        n0 = s * NS
        # fetch per device shard: skips the nd-shard gather copy that a
        # full np.asarray of the sharded array would do
        for sh in y.addressable_shards:
            i = sh.index[0].start // R_OUT if sh.index else 0
            b = batches[i]
            yh = np.asarray(sh.data)
            yb = yh[0:R_X, :].reshape(L, NS, CH)
            oscb = yh[R_X:R_OUT, :].view(np.float16)
            sf = (
                oscb.reshape(C, NS, NC)
                .transpose(2, 0, 1)
                .reshape(L, NS)
                .astype(np.float32)
            )
            np.multiply(yb, sf[:, :, None], out=res[b, :, n0:n0 + NS, :])

    return put_weights, dispatch, fetch_dequant


_RUNNER = None


def _make_runner():
    import jax

    put_weights, dispatch, fetch_dequant = _make_io(jax.devices()[:8])
    batches = list(range(8))
    # persistent host buffers, reused across calls to avoid page-fault
    # churn on the single CPU
    blobs = [np.empty((8 * R_IN, F), np.int8) for _ in range(S)]
    qscratch = np.empty((L, NS, F), np.float32)
    res_buf = np.empty((8, L, NSEQ, CH), np.float32)

    def call(query, Wq, bq_, Wk, bk_, Wv, bv_):
        query = np.asarray(query, np.float32)
        # weights go up async, hidden under the slice-0 quantization
        wdev = put_weights(_pack_weights(8, Wq, bq_, Wk, bk_, Wv, bv_))
        ys = []
        for s in range(S):
            for i in range(8):
                _pack_batch(blobs[s], i, query[i], s, qscratch)
            ys.append(dispatch(blobs[s], wdev))
        for s in range(S):
            fetch_dequant(ys[s], s, batches, res_buf)
        return res_buf

    # Warm the transfer path (TCP windows, jit caches, device allocs) so the
    # first real call runs at steady-state speed. Random data keeps the wire
    # byte profile realistic for the tunnel's compressor.
    rng = np.random.default_rng(0)
    dq = rng.standard_normal((8, L, NSEQ, F)).astype(np.float32)
    dw = rng.standard_normal((F, CH)).astype(np.float32) * np.float32(0.09)
    db = np.zeros((CH,), np.float32)
    for _ in range(2):
        call(dq, dw, db, dw, db, dw, db)

    # long-lived state out of the gc generations: avoids collector scans
    # pausing the single CPU mid-pipeline
    import gc

    gc.collect()
    gc.freeze()

    return call


def kernel(query, Wq, bq, Wk, bk, Wv, bv):
    global _RUNNER
    if _RUNNER is None:
        _RUNNER = _make_runner()
    return _RUNNER(np.asarray(query, np.float32), Wq, bq, Wk, bk, Wv, bv)
